# revision 26
# baseline (speedup 1.0000x reference)
"""Adaptive average pooling (8,384,384,64) NHWC -> (8,7,7,64) on 8 TRN2 NeuronCores.

Pure data parallel: one batch sample per core, no collectives. Per core:
  - W is covered by 5 nearly non-overlapping spans [0,110) [110,220)
    [220,275) [275,330) [329,384) (only column 329 is read twice). The
    boundary columns 109/219/274 that adaptive windows 2/4/5 need are
    patched into a pad column slot by the ACT engine from the previous
    span's resident slab instead of being re-read from HBM.
  - Span 4's H-chunk 0 streams f32 over the sync HWDGE ring at block
    start (the SWDGE Q7 needs ~3us of init before its first
    descriptors) and DVE downcasts it; the other 14 slabs stream via
    SWDGE DMAs that cast f32 -> bf16 in flight, alternating two SWDGE
    queues over a 6-slot SBUF ring. Span 3's H-chunk 0 and 2 slabs
    each stream as two pieces so the tail window's matmuls pipeline
    with their arrival.
  - SDMA engine 15 runs ~20% slower than engines 0-13 under SWDGE
    load (descriptor-ring AXI port contention), and the slab
    completion semaphores make every slab wait for it, so the whole
    pipeline throttles to its pace through ring-WAR backpressure.
    Rebalance: SWDGE descriptors deal per ROW round-robin over the 16
    engines with a persistent ring pointer (R%16==0 transfers are
    uniform and pointer-neutral; an R<16-row runt covers engines
    P..P+R-1 and advances P by R). Two slabs are emitted as
    full-starve patterns -- 8 x ([14-row runt to engines 0-13] +
    [2-row 64B shim to engines 14/15]) + [16-row uniform],
    pointer-neutral, alternating the two SWDGE queues per transfer --
    so engines 14/15 keep ~0.85 of a fair byte share, matching their
    slowness so all 16 engines finish streaming together. Deeper
    starving backfires: each extra runt transfer costs every fast
    engine a ~0.7us completion-descriptor write-ack flush.
  - The pmat const loads via the ACT HWDGE ring.
  - TensorEngine reduces over H (the partition dim) with bf16 matmuls:
    stationary P_{j,k} (128 x 7) is a bf16 membership mask of the
    H-windows scaled by 1/(sh_i*sw_j), so no separate averaging pass is
    needed. Each window j owns ONE PSUM bank: all 21 matmuls (3 H-chunks
    x 7 moving-operand chunks of <=512 cols) accumulate into it, so the
    chunk dimension folds inside PSUM and windows never share banks --
    no PE<->DVE write-after-read chain between windows.
  - DVE does ONE op per window right after its stop-matmul: a strided
    X-reduce of the 512-col bank over w' straight into the output tile.
    The first six windows' results go out on sync right after window 4
    finishes; the tail window's 64 channels go out in a final small DMA.

Raw Bass blocks with explicit semaphores (TileContext's generated sync
exceeds this toolchain's per-instruction sync-wait limits).
"""

import numpy as np
import ml_dtypes

import concourse.bass as bass
import concourse.mybir as mybir
from concourse.bass_utils import run_bass_kernel_spmd

B, H, W, C = 8, 384, 384, 64
OUT = 7
N_CORES = 8
KH = H // 128  # 3 H-chunks of 128 rows
NCH = 7  # moving-operand chunks per window
# (first W column, data width, data offset inside the ring slot)
SPANS = [(0, 110, 0), (110, 110, 1), (220, 55, 1), (275, 55, 1), (329, 55, 0)]
STREAM = [4, 0, 1, 2, 3]  # span stream order (span 4 first: window 6 runs first)
NSW = 15  # slabs: 5 spans x 3 H-chunks (slab 0 goes via sync+DVE, not SWDGE)
SLOT = 111 * C  # ring slot size in elements
RING = 6  # slab ring depth
W512 = (512,) * 7
W448 = (512,) * 6 + (448,)
# (span g, view offset in slot cols, per-chunk widths)
WINDOWS = [
    (0, 0, W448),
    (0, 54, W512),
    (1, 0, W512),
    (1, 55, W512),
    (2, 0, W512),
    (3, 0, W512),
    (4, 0, W448),
]
FIRST_WIN = {4: 6, 0: 0, 1: 2, 2: 4, 3: 5}  # span -> its first window in WORDER
LAST_WIN = {4: 6, 0: 1, 1: 3, 2: 4, 3: 5}  # span -> its last window in WORDER
WORDER = [6, 0, 1, 2, 3, 4, 5]  # PE/DVE window processing order
# boundary-column patches: (src span, src slot col, dst span, WAR order idx)
PATCHES = [
    (0, 109, 1, 0),  # W109 for window 2: span0 col109 -> span1 pad, after w6
    (1, 110, 2, 1),  # W219 for window 4: span1 col110 -> span2 pad, after w0
    (2, 55, 3, 3),  # W274 for window 5: span2 col55 -> span3 pad, after w2
]
PAD_OF_WIN = {2: 0, 4: 1, 5: 2}  # window -> patch entry guarding its chunk 0

STARVE = (4, 10)  # full-starve slabs: span0 k1 (110-col) + span2 k1 (55-col)

_F32 = mybir.dt.float32
_BF16 = mybir.dt.bfloat16

SEM_TGT = [272 if _i in STARVE else 16 for _i in range(15)]
SLAB_OF = {}
for _i in range(NSW):
    SLAB_OF[(STREAM[_i // KH], _i % KH)] = _i
SPAN_OF_SLAB = [STREAM[_i // KH] for _i in range(NSW)]
SPLIT = (12, 14)  # span-3 k0/k2 slabs stream as two pieces
SPA = 4 * 512  # piece split point (view elems): chunks 0-3 | 4-6


def _windows(d, out):
    starts = np.floor(np.arange(out) * d / out).astype(np.int64)
    ends = np.ceil((np.arange(out) + 1) * d / out).astype(np.int64)
    return starts, ends - starts


def _build():
    nc = bass.Bass(num_swdge_queues=2)
    x = nc.declare_dram_parameter("x", [H, W * C], _F32, isOutput=False)
    pmat = nc.declare_dram_parameter(
        "pmat", [128, OUT * KH * OUT], _BF16, isOutput=False
    )
    out = nc.declare_dram_parameter("out", [OUT, OUT * C], _F32, isOutput=True)

    with (
        nc.sbuf_tensor([128, RING * SLOT], _BF16) as xbuf,
        nc.sbuf_tensor([128, 55 * C], _F32) as stage,
        nc.sbuf_tensor([128, 55 * C], _F32) as stage1,
        nc.sbuf_tensor([128, 55 * C], _F32) as stage2,
        nc.sbuf_tensor([16, 16], _F32) as shim_sb,
        nc.sbuf_tensor([128, OUT * KH * OUT], _BF16) as p_sb,
        nc.sbuf_tensor([OUT, OUT * C], _F32) as y_sb,
        nc.psum_tensor([128, OUT * 512], _F32) as psum,
        nc.semaphore("const_sem") as const_sem,
        nc.semaphore("warm_sem") as warm_sem,
        nc.semaphore("stage_sem") as stage_sem,
        nc.semaphore("stage1_sem") as stage1_sem,
        nc.semaphore("stage2_sem") as stage2_sem,
        nc.semaphore("pad_sem") as pad_sem,
        nc.semaphore("pe_pass_sem") as pe_pass_sem,
        nc.semaphore("win_sem") as win_sem,
        nc.semaphore("mul_sem") as mul_sem,
        nc.semaphore("out_sem") as out_sem,
    ):
        slab_sems = [nc.alloc_semaphore(f"slab{i}") for i in range(NSW)]
        pieceb_sems = {i: nc.alloc_semaphore(f"slab{i}b") for i in SPLIT}

        ORDER_OF = {j: o for o, j in enumerate(WORDER)}

        def pass_wait(eng, o, k):
            """Wait until the PE finished pass (order-index o, H-chunk k)."""
            if k == KH - 1:
                eng.wait_ge(win_sem, o + 1)
            else:
                eng.wait_ge(pe_pass_sem, o * (KH - 1) + k + 1)

        def slot_col(i, col):
            return (i % RING) * SLOT + col * C

        def slot_war(eng, i):
            """Ring-slot WAR: previous occupant's windows must be done."""
            if i >= RING:
                gp, kp = SPAN_OF_SLAB[i - RING], (i - RING) % KH
                pass_wait(eng, ORDER_OF[LAST_WIN[gp]], kp)

        with nc.Block(no_gpsimd_drain=True) as block:

            @block.gpsimd
            def _(gpsimd):
                for i in range(3, NSW):
                    g, k = SPAN_OF_SLAB[i], i % KH
                    w0, wd, doff = SPANS[g]
                    slot_war(gpsimd, i)
                    if i in STARVE:
                        # full-starve: 8x(14-runt to engines 0-13 + 2-shim
                        # to engines 14/15) + 16-row uniform; consecutive
                        # transfers alternate SWDGE queues so each engine
                        # pipelines across transfer boundaries
                        qq = [0]

                        def q_alt(dma):
                            if qq[0] % 2 == 1:
                                dma.ins.queue = "qPoolDynamic1"
                            qq[0] += 1

                        for jj in range(8):
                            r0 = 14 * jj
                            q_alt(
                                gpsimd.dma_start(
                                    out=xbuf[
                                        r0 : r0 + 14,
                                        slot_col(i, doff) : slot_col(
                                            i, doff + wd
                                        ),
                                    ],
                                    in_=x[
                                        k * 128 + r0 : k * 128 + r0 + 14,
                                        w0 * C : (w0 + wd) * C,
                                    ],
                                ).then_inc(slab_sems[i], 16)
                            )
                            q_alt(
                                gpsimd.dma_start(
                                    out=shim_sb[0:2, 0:8], in_=x[0:2, 0:8]
                                ).then_inc(slab_sems[i], 16)
                            )
                        q_alt(
                            gpsimd.dma_start(
                                out=xbuf[
                                    112:128,
                                    slot_col(i, doff) : slot_col(i, doff + wd),
                                ],
                                in_=x[
                                    k * 128 + 112 : (k + 1) * 128,
                                    w0 * C : (w0 + wd) * C,
                                ],
                            ).then_inc(slab_sems[i], 16)
                        )
                        continue
                    if i in SPLIT:
                        gpsimd.dma_start(
                            out=xbuf[
                                :, slot_col(i, doff) : slot_col(i, 0) + SPA
                            ],
                            in_=x[
                                k * 128 : (k + 1) * 128,
                                w0 * C : w0 * C + SPA - doff * C,
                            ],
                        ).then_inc(slab_sems[i], 16)
                        gpsimd.dma_start(
                            out=xbuf[
                                :,
                                slot_col(i, 0) + SPA : slot_col(i, doff)
                                + wd * C,
                            ],
                            in_=x[
                                k * 128 : (k + 1) * 128,
                                w0 * C + SPA - doff * C : (w0 + wd) * C,
                            ],
                        ).then_inc(pieceb_sems[i], 16)
                        continue
                    dma = gpsimd.dma_start(
                        out=xbuf[:, slot_col(i, doff) : slot_col(i, doff + wd)],
                        in_=x[k * 128 : (k + 1) * 128, w0 * C : (w0 + wd) * C],
                    ).then_inc(slab_sems[i], 16)
                    if i % 2 == 1:
                        dma.ins.queue = "qPoolDynamic1"

            @block.sync
            def _(sync):
                # span 4 H-chunk 0 as f32 while the SWDGE Q7 initializes
                w0, wd, _ = SPANS[4]
                sync.dma_start(
                    out=stage[:], in_=x[0:128, w0 * C : (w0 + wd) * C]
                ).then_inc(stage_sem, 16)
                sync.dma_start(
                    out=stage2[:], in_=x[256:384, w0 * C : (w0 + wd) * C]
                ).then_inc(stage2_sem, 16)
                # windows 0-4 and 6 are final once window 4 (order 5) scaled
                sync.wait_ge(mul_sem, 1)
                sync.dma_start(
                    out=out[:, 0 : 5 * C], in_=y_sb[:, 0 : 5 * C]
                ).then_inc(out_sem, 16)
                sync.dma_start(
                    out=out[:, 6 * C : 7 * C], in_=y_sb[:, 6 * C : 7 * C]
                ).then_inc(out_sem, 16)
                # tail window 5
                sync.wait_ge(mul_sem, 2)
                sync.dma_start(
                    out=out[:, 5 * C : 6 * C], in_=y_sb[:, 5 * C : 6 * C]
                ).then_inc(out_sem, 16)
                sync.wait_ge(out_sem, 48)

            @block.scalar
            def _(scalar):
                scalar.dma_start(out=p_sb[:], in_=pmat[:]).then_inc(const_sem, 16)
                w4, wd4, _ = SPANS[4]
                scalar.dma_start(
                    out=stage1[:], in_=x[128:256, w4 * C : (w4 + wd4) * C]
                ).then_inc(stage1_sem, 16)
                for ent, (src_g, src_col, dst_g, war_o) in enumerate(PATCHES):
                    for k in range(KH):
                        si = SLAB_OF[(src_g, k)]
                        di = SLAB_OF[(dst_g, k)]
                        if si == 0:
                            scalar.wait_ge(warm_sem, 1)
                        else:
                            scalar.wait_ge(slab_sems[si], SEM_TGT[si])
                        # WAR: the pad column slot still holds data the
                        # previous occupant's windows read
                        pass_wait(scalar, war_o, k)
                        scalar.copy(
                            xbuf[:, slot_col(di, 0) : slot_col(di, 1)],
                            xbuf[
                                :,
                                slot_col(si, src_col) : slot_col(
                                    si, src_col + 1
                                ),
                            ],
                        ).then_inc(pad_sem, 1)

            @block.tensor
            def _(tensor):
                tensor.wait_ge(const_sem, 16)
                for o, j in enumerate(WORDER):
                    g, off, widths = WINDOWS[j]
                    for k in range(KH):
                        i = SLAB_OF[(g, k)]
                        if j == FIRST_WIN[g]:
                            if i == 0:
                                tensor.wait_ge(warm_sem, 1)
                            else:
                                tensor.wait_ge(slab_sems[i], SEM_TGT[i])
                        if j in PAD_OF_WIN:
                            tensor.wait_ge(
                                pad_sem, PAD_OF_WIN[j] * KH + k + 1
                            )
                        base = slot_col(i, off)
                        n = j * KH + k
                        lhsT = p_sb[:, n * OUT : (n + 1) * OUT]
                        for cb in range(NCH):
                            if i in SPLIT and cb == 4:
                                tensor.wait_ge(pieceb_sems[i], 16)
                            mm = tensor.matmul(
                                psum[:OUT, j * 512 : j * 512 + widths[cb]],
                                lhsT,
                                xbuf[
                                    :,
                                    base + cb * 512 : base
                                    + cb * 512
                                    + widths[cb],
                                ],
                                start=(k == 0 and cb == 0),
                                stop=(k == KH - 1 and cb == NCH - 1),
                            )
                        if k == KH - 1:
                            mm.then_inc(win_sem, 1)
                        else:
                            mm.then_inc(pe_pass_sem, 1)

            @block.vector
            def _(vector):
                # downcast the sync-streamed span4 H-chunk 0 into ring slot 0
                vector.wait_ge(stage_sem, 16)
                vector.tensor_copy(xbuf[:, 0 : 55 * C], stage[:]).then_inc(
                    warm_sem, 1
                )
                vector.wait_ge(stage1_sem, 16)
                vector.tensor_copy(
                    xbuf[:, SLOT : SLOT + 55 * C], stage1[:]
                ).then_inc(slab_sems[1], 16)
                vector.wait_ge(stage2_sem, 16)
                vector.tensor_copy(
                    xbuf[:, 2 * SLOT : 2 * SLOT + 55 * C], stage2[:]
                ).then_inc(slab_sems[2], 16)
                for o, j in enumerate(WORDER):
                    vector.wait_ge(win_sem, o + 1)
                    # fold the bank's 8 w' column groups into the window
                    # average (the 1/(sh*sw) scale is baked into pmat)
                    red = vector.tensor_reduce(
                        out=y_sb[:, j * C : (j + 1) * C],
                        in_=psum[:OUT, j * 512 : (j + 1) * 512].rearrange(
                            "p (w c) -> p c w", c=C
                        ),
                        axis=mybir.AxisListType.X,
                        op=mybir.AluOpType.add,
                    )
                    if o >= OUT - 2:
                        red.then_inc(mul_sem, 1)

    return nc


def _consts():
    hs, hsz = _windows(H, OUT)
    _, wsz = _windows(W, OUT)
    p = np.zeros((128, OUT * KH * OUT), np.float32)
    for j in range(OUT):
        for k in range(KH):
            n = j * KH + k
            for i in range(OUT):
                h0, h1 = int(hs[i]), int(hs[i] + hsz[i])
                for h in range(max(h0, k * 128), min(h1, (k + 1) * 128)):
                    p[h - k * 128, n * OUT + i] = 1.0 / (
                        float(hsz[i]) * float(wsz[j])
                    )
    return p.astype(ml_dtypes.bfloat16)


_NC_CACHE = None


def _run(x, **kwargs):
    global _NC_CACHE
    if _NC_CACHE is None:
        _NC_CACHE = _build()
    nc = _NC_CACHE
    p = _consts()
    x = np.ascontiguousarray(np.asarray(x, dtype=np.float32))
    in_maps = [
        {"x": x[b].reshape(H, W * C), "pmat": p}
        for b in range(N_CORES)
    ]
    res = run_bass_kernel_spmd(nc, in_maps, core_ids=list(range(N_CORES)), **kwargs)
    y = np.stack(
        [res.results[b]["out"].reshape(OUT, OUT, C) for b in range(N_CORES)]
    )
    return y, res


def kernel(x: np.ndarray) -> np.ndarray:
    y, _ = _run(x)
    return y


# revision 27
# speedup vs baseline: 1.0963x; 1.0963x over previous
"""Adaptive average pooling (8,384,384,64) NHWC -> (8,7,7,64) on 8 TRN2 NeuronCores.

Pure data parallel: one batch sample per core, no collectives. Per core:
  - W is covered by 5 nearly non-overlapping spans [0,110) [110,220)
    [220,275) [275,330) [329,384) (only column 329 is read twice). The
    boundary columns 109/219/274 that adaptive windows 2/4/5 need are
    patched into a pad column slot by the ACT engine from the previous
    span's resident slab instead of being re-read from HBM.
  - Span 4's H-chunk 0 streams f32 over the sync HWDGE ring at block
    start (the SWDGE Q7 needs ~3us of init before its first
    descriptors) and DVE downcasts it; the other 14 slabs stream via
    SWDGE DMAs that cast f32 -> bf16 in flight, alternating two SWDGE
    queues over a 6-slot SBUF ring. Span 3's H-chunk 0 and 2 slabs
    each stream as two pieces so the tail window's matmuls pipeline
    with their arrival.
  - SDMA engine 15 runs ~20% slower than engines 0-13 under SWDGE
    load (descriptor-ring AXI port contention), and the slab
    completion semaphores make every slab wait for it, so the whole
    pipeline throttles to its pace through ring-WAR backpressure.
    Rebalance: SWDGE descriptors deal per ROW round-robin over the 16
    engines with a persistent ring pointer (R%16==0 transfers are
    uniform and pointer-neutral; an R<16-row runt covers engines
    P..P+R-1 and advances P by R). Two slabs are emitted as
    full-starve patterns -- 8 x ([14-row runt to engines 0-13] +
    [2-row 64B shim to engines 14/15]) + [16-row uniform],
    pointer-neutral, alternating the two SWDGE queues per transfer --
    so engines 14/15 keep ~0.85 of a fair byte share, matching their
    slowness so all 16 engines finish streaming together. Deeper
    starving backfires: each extra runt transfer costs every fast
    engine a ~0.7us completion-descriptor write-ack flush.
  - The pmat const loads via the ACT HWDGE ring.
  - TensorEngine reduces over H (the partition dim) with bf16 matmuls:
    stationary P_{j,k} (128 x 7) is a bf16 membership mask of the
    H-windows scaled by 1/(sh_i*sw_j), so no separate averaging pass is
    needed. Each window j owns ONE PSUM bank: all 21 matmuls (3 H-chunks
    x 7 moving-operand chunks of <=512 cols) accumulate into it, so the
    chunk dimension folds inside PSUM and windows never share banks --
    no PE<->DVE write-after-read chain between windows.
  - DVE does ONE op per window right after its stop-matmul: a strided
    X-reduce of the 512-col bank over w' straight into the output tile.
    The first six windows' results go out on sync right after window 4
    finishes; the tail window's 64 channels go out in a final small DMA.

Raw Bass blocks with explicit semaphores (TileContext's generated sync
exceeds this toolchain's per-instruction sync-wait limits).
"""

import numpy as np
import ml_dtypes

import concourse.bass as bass
import concourse.mybir as mybir
from concourse.bass_utils import run_bass_kernel_spmd

B, H, W, C = 8, 384, 384, 64
OUT = 7
N_CORES = 8
KH = H // 128  # 3 H-chunks of 128 rows
NCH = 7  # moving-operand chunks per window
# (first W column, data width, data offset inside the ring slot)
SPANS = [(0, 110, 0), (110, 110, 1), (220, 55, 1), (275, 55, 1), (329, 55, 0)]
STREAM = [4, 0, 1, 2, 3]  # span stream order (span 4 first: window 6 runs first)
NSW = 15  # slabs: 5 spans x 3 H-chunks (slab 0 goes via sync+DVE, not SWDGE)
SLOT = 111 * C  # ring slot size in elements
RING = 6  # slab ring depth
W512 = (512,) * 7
W448 = (512,) * 6 + (448,)
# (span g, view offset in slot cols, per-chunk widths)
WINDOWS = [
    (0, 0, W448),
    (0, 54, W512),
    (1, 0, W512),
    (1, 55, W512),
    (2, 0, W512),
    (3, 0, W512),
    (4, 0, W448),
]
FIRST_WIN = {4: 6, 0: 0, 1: 2, 2: 4, 3: 5}  # span -> its first window in WORDER
LAST_WIN = {4: 6, 0: 1, 1: 3, 2: 4, 3: 5}  # span -> its last window in WORDER
WORDER = [6, 0, 1, 2, 3, 4, 5]  # PE/DVE window processing order
# boundary-column patches: (src span, src slot col, dst span, WAR order idx)
PATCHES = [
    (0, 109, 1, 0),  # W109 for window 2: span0 col109 -> span1 pad, after w6
    (1, 110, 2, 1),  # W219 for window 4: span1 col110 -> span2 pad, after w0
    (2, 55, 3, 3),  # W274 for window 5: span2 col55 -> span3 pad, after w2
]
PAD_OF_WIN = {2: 0, 4: 1, 5: 2}  # window -> patch entry guarding its chunk 0

STARVE = (4, 10)  # full-starve slabs: span0 k1 (110-col) + span2 k1 (55-col)

_F32 = mybir.dt.float32
_BF16 = mybir.dt.bfloat16

SEM_TGT = [272 if _i in STARVE else 16 for _i in range(15)]
SLAB_OF = {}
for _i in range(NSW):
    SLAB_OF[(STREAM[_i // KH], _i % KH)] = _i
SPAN_OF_SLAB = [STREAM[_i // KH] for _i in range(NSW)]
SPLIT = (12, 14)  # span-3 k0/k2 slabs stream as two pieces
SPA = 4 * 512  # piece split point (view elems): chunks 0-3 | 4-6


def _windows(d, out):
    starts = np.floor(np.arange(out) * d / out).astype(np.int64)
    ends = np.ceil((np.arange(out) + 1) * d / out).astype(np.int64)
    return starts, ends - starts


def _build():
    nc = bass.Bass(num_swdge_queues=2)
    x = nc.declare_dram_parameter("x", [H, W * C], _F32, isOutput=False)
    pmat = nc.declare_dram_parameter(
        "pmat", [128, OUT * KH * OUT], _BF16, isOutput=False
    )
    out = nc.declare_dram_parameter("out", [OUT, OUT * C], _F32, isOutput=True)

    with (
        nc.sbuf_tensor([128, RING * SLOT], _BF16) as xbuf,
        nc.sbuf_tensor([128, 55 * C], _F32) as stage,
        nc.sbuf_tensor([128, 55 * C], _F32) as stage1,
        nc.sbuf_tensor([16, 16], _F32) as shim_sb,
        nc.sbuf_tensor([128, OUT * KH * OUT], _BF16) as p_sb,
        nc.sbuf_tensor([OUT, OUT * C], _F32) as y_sb,
        nc.psum_tensor([128, OUT * 512], _F32) as psum,
        nc.semaphore("const_sem") as const_sem,
        nc.semaphore("warm_sem") as warm_sem,
        nc.semaphore("stage_sem") as stage_sem,
        nc.semaphore("stage1_sem") as stage1_sem,
        nc.semaphore("pad_sem") as pad_sem,
        nc.semaphore("pe_pass_sem") as pe_pass_sem,
        nc.semaphore("win_sem") as win_sem,
        nc.semaphore("mul_sem") as mul_sem,
        nc.semaphore("out_sem") as out_sem,
    ):
        slab_sems = [nc.alloc_semaphore(f"slab{i}") for i in range(NSW)]
        pieceb_sems = {i: nc.alloc_semaphore(f"slab{i}b") for i in SPLIT}

        ORDER_OF = {j: o for o, j in enumerate(WORDER)}

        def pass_wait(eng, o, k):
            """Wait until the PE finished pass (order-index o, H-chunk k)."""
            if k == KH - 1:
                eng.wait_ge(win_sem, o + 1)
            else:
                eng.wait_ge(pe_pass_sem, o * (KH - 1) + k + 1)

        def slot_col(i, col):
            return (i % RING) * SLOT + col * C

        def slot_war(eng, i):
            """Ring-slot WAR: previous occupant's windows must be done."""
            if i >= RING:
                gp, kp = SPAN_OF_SLAB[i - RING], (i - RING) % KH
                pass_wait(eng, ORDER_OF[LAST_WIN[gp]], kp)

        with nc.Block(no_gpsimd_drain=True) as block:

            @block.gpsimd
            def _(gpsimd):
                for i in range(2, NSW):
                    g, k = SPAN_OF_SLAB[i], i % KH
                    w0, wd, doff = SPANS[g]
                    slot_war(gpsimd, i)
                    if i in STARVE:
                        # full-starve: 8x(14-runt to engines 0-13 + 2-shim
                        # to engines 14/15) + 16-row uniform; consecutive
                        # transfers alternate SWDGE queues so each engine
                        # pipelines across transfer boundaries
                        qq = [0]

                        def q_alt(dma):
                            if qq[0] % 2 == 1:
                                dma.ins.queue = "qPoolDynamic1"
                            qq[0] += 1

                        for jj in range(8):
                            r0 = 14 * jj
                            q_alt(
                                gpsimd.dma_start(
                                    out=xbuf[
                                        r0 : r0 + 14,
                                        slot_col(i, doff) : slot_col(
                                            i, doff + wd
                                        ),
                                    ],
                                    in_=x[
                                        k * 128 + r0 : k * 128 + r0 + 14,
                                        w0 * C : (w0 + wd) * C,
                                    ],
                                ).then_inc(slab_sems[i], 16)
                            )
                            q_alt(
                                gpsimd.dma_start(
                                    out=shim_sb[0:2, 0:8], in_=x[0:2, 0:8]
                                ).then_inc(slab_sems[i], 16)
                            )
                        q_alt(
                            gpsimd.dma_start(
                                out=xbuf[
                                    112:128,
                                    slot_col(i, doff) : slot_col(i, doff + wd),
                                ],
                                in_=x[
                                    k * 128 + 112 : (k + 1) * 128,
                                    w0 * C : (w0 + wd) * C,
                                ],
                            ).then_inc(slab_sems[i], 16)
                        )
                        continue
                    if i in SPLIT:
                        gpsimd.dma_start(
                            out=xbuf[
                                :, slot_col(i, doff) : slot_col(i, 0) + SPA
                            ],
                            in_=x[
                                k * 128 : (k + 1) * 128,
                                w0 * C : w0 * C + SPA - doff * C,
                            ],
                        ).then_inc(slab_sems[i], 16)
                        gpsimd.dma_start(
                            out=xbuf[
                                :,
                                slot_col(i, 0) + SPA : slot_col(i, doff)
                                + wd * C,
                            ],
                            in_=x[
                                k * 128 : (k + 1) * 128,
                                w0 * C + SPA - doff * C : (w0 + wd) * C,
                            ],
                        ).then_inc(pieceb_sems[i], 16)
                        continue
                    dma = gpsimd.dma_start(
                        out=xbuf[:, slot_col(i, doff) : slot_col(i, doff + wd)],
                        in_=x[k * 128 : (k + 1) * 128, w0 * C : (w0 + wd) * C],
                    ).then_inc(slab_sems[i], 16)
                    if i % 2 == 1:
                        dma.ins.queue = "qPoolDynamic1"

            @block.sync
            def _(sync):
                # span 4 H-chunk 0 as f32 while the SWDGE Q7 initializes
                w0, wd, _ = SPANS[4]
                sync.dma_start(
                    out=stage[:], in_=x[0:128, w0 * C : (w0 + wd) * C]
                ).then_inc(stage_sem, 16)
                # windows 0-4 and 6 are final once window 4 (order 5) scaled
                sync.wait_ge(mul_sem, 1)
                sync.dma_start(
                    out=out[:, 0 : 5 * C], in_=y_sb[:, 0 : 5 * C]
                ).then_inc(out_sem, 16)
                sync.dma_start(
                    out=out[:, 6 * C : 7 * C], in_=y_sb[:, 6 * C : 7 * C]
                ).then_inc(out_sem, 16)
                # tail window 5
                sync.wait_ge(mul_sem, 2)
                sync.dma_start(
                    out=out[:, 5 * C : 6 * C], in_=y_sb[:, 5 * C : 6 * C]
                ).then_inc(out_sem, 16)
                sync.wait_ge(out_sem, 48)

            @block.scalar
            def _(scalar):
                scalar.dma_start(out=p_sb[:], in_=pmat[:]).then_inc(const_sem, 16)
                w4, wd4, _ = SPANS[4]
                scalar.dma_start(
                    out=stage1[:], in_=x[128:256, w4 * C : (w4 + wd4) * C]
                ).then_inc(stage1_sem, 16)
                scalar.wait_ge(stage1_sem, 16)
                scalar.copy(
                    xbuf[:, SLOT : SLOT + 55 * C], stage1[:]
                ).then_inc(slab_sems[1], 16)
                for ent, (src_g, src_col, dst_g, war_o) in enumerate(PATCHES):
                    for k in range(KH):
                        si = SLAB_OF[(src_g, k)]
                        di = SLAB_OF[(dst_g, k)]
                        if si == 0:
                            scalar.wait_ge(warm_sem, 1)
                        else:
                            scalar.wait_ge(slab_sems[si], SEM_TGT[si])
                        # WAR: the pad column slot still holds data the
                        # previous occupant's windows read
                        pass_wait(scalar, war_o, k)
                        scalar.copy(
                            xbuf[:, slot_col(di, 0) : slot_col(di, 1)],
                            xbuf[
                                :,
                                slot_col(si, src_col) : slot_col(
                                    si, src_col + 1
                                ),
                            ],
                        ).then_inc(pad_sem, 1)

            @block.tensor
            def _(tensor):
                tensor.wait_ge(const_sem, 16)
                for o, j in enumerate(WORDER):
                    g, off, widths = WINDOWS[j]
                    for k in range(KH):
                        i = SLAB_OF[(g, k)]
                        if j == FIRST_WIN[g]:
                            if i == 0:
                                tensor.wait_ge(warm_sem, 1)
                            else:
                                tensor.wait_ge(slab_sems[i], SEM_TGT[i])
                        if j in PAD_OF_WIN:
                            tensor.wait_ge(
                                pad_sem, PAD_OF_WIN[j] * KH + k + 1
                            )
                        base = slot_col(i, off)
                        n = j * KH + k
                        lhsT = p_sb[:, n * OUT : (n + 1) * OUT]
                        for cb in range(NCH):
                            if i in SPLIT and cb == 4:
                                tensor.wait_ge(pieceb_sems[i], 16)
                            mm = tensor.matmul(
                                psum[:OUT, j * 512 : j * 512 + widths[cb]],
                                lhsT,
                                xbuf[
                                    :,
                                    base + cb * 512 : base
                                    + cb * 512
                                    + widths[cb],
                                ],
                                start=(k == 0 and cb == 0),
                                stop=(k == KH - 1 and cb == NCH - 1),
                            )
                        if k == KH - 1:
                            mm.then_inc(win_sem, 1)
                        else:
                            mm.then_inc(pe_pass_sem, 1)

            @block.vector
            def _(vector):
                # downcast the sync-streamed span4 H-chunk 0 into ring slot 0
                vector.wait_ge(stage_sem, 16)
                vector.tensor_copy(xbuf[:, 0 : 55 * C], stage[:]).then_inc(
                    warm_sem, 1
                )
                for o, j in enumerate(WORDER):
                    vector.wait_ge(win_sem, o + 1)
                    # fold the bank's 8 w' column groups into the window
                    # average (the 1/(sh*sw) scale is baked into pmat)
                    red = vector.tensor_reduce(
                        out=y_sb[:, j * C : (j + 1) * C],
                        in_=psum[:OUT, j * 512 : (j + 1) * 512].rearrange(
                            "p (w c) -> p c w", c=C
                        ),
                        axis=mybir.AxisListType.X,
                        op=mybir.AluOpType.add,
                    )
                    if o >= OUT - 2:
                        red.then_inc(mul_sem, 1)

    return nc


def _consts():
    hs, hsz = _windows(H, OUT)
    _, wsz = _windows(W, OUT)
    p = np.zeros((128, OUT * KH * OUT), np.float32)
    for j in range(OUT):
        for k in range(KH):
            n = j * KH + k
            for i in range(OUT):
                h0, h1 = int(hs[i]), int(hs[i] + hsz[i])
                for h in range(max(h0, k * 128), min(h1, (k + 1) * 128)):
                    p[h - k * 128, n * OUT + i] = 1.0 / (
                        float(hsz[i]) * float(wsz[j])
                    )
    return p.astype(ml_dtypes.bfloat16)


_NC_CACHE = None


def _run(x, **kwargs):
    global _NC_CACHE
    if _NC_CACHE is None:
        _NC_CACHE = _build()
    nc = _NC_CACHE
    p = _consts()
    x = np.ascontiguousarray(np.asarray(x, dtype=np.float32))
    in_maps = [
        {"x": x[b].reshape(H, W * C), "pmat": p}
        for b in range(N_CORES)
    ]
    res = run_bass_kernel_spmd(nc, in_maps, core_ids=list(range(N_CORES)), **kwargs)
    y = np.stack(
        [res.results[b]["out"].reshape(OUT, OUT, C) for b in range(N_CORES)]
    )
    return y, res


def kernel(x: np.ndarray) -> np.ndarray:
    y, _ = _run(x)
    return y


# revision 28
# speedup vs baseline: 1.1058x; 1.0087x over previous
"""Adaptive average pooling (8,384,384,64) NHWC -> (8,7,7,64) on 8 TRN2 NeuronCores.

Pure data parallel: one batch sample per core, no collectives. Per core:
  - W is covered by 5 nearly non-overlapping spans [0,110) [110,220)
    [220,275) [275,330) [329,384) (only column 329 is read twice). The
    boundary columns 109/219/274 that adaptive windows 2/4/5 need are
    patched into a pad column slot by the ACT engine from the previous
    span's resident slab instead of being re-read from HBM.
  - Span 4's H-chunk 0 streams f32 over the sync HWDGE ring at block
    start (the SWDGE Q7 needs ~3us of init before its first
    descriptors) and DVE downcasts it; the other 14 slabs stream via
    SWDGE DMAs that cast f32 -> bf16 in flight, alternating two SWDGE
    queues over a 6-slot SBUF ring. Span 3's H-chunk 0 and 2 slabs
    each stream as two pieces so the tail window's matmuls pipeline
    with their arrival.
  - SDMA engine 15 runs ~20% slower than engines 0-13 under SWDGE
    load (descriptor-ring AXI port contention), and the slab
    completion semaphores make every slab wait for it, so the whole
    pipeline throttles to its pace through ring-WAR backpressure.
    Rebalance: SWDGE descriptors deal per ROW round-robin over the 16
    engines with a persistent ring pointer (R%16==0 transfers are
    uniform and pointer-neutral; an R<16-row runt covers engines
    P..P+R-1 and advances P by R). Two slabs are emitted as
    full-starve patterns -- 8 x ([14-row runt to engines 0-13] +
    [2-row 64B shim to engines 14/15]) + [16-row uniform],
    pointer-neutral, alternating the two SWDGE queues per transfer --
    so engines 14/15 keep ~0.85 of a fair byte share, matching their
    slowness so all 16 engines finish streaming together. Deeper
    starving backfires: each extra runt transfer costs every fast
    engine a ~0.7us completion-descriptor write-ack flush.
  - The pmat const loads via the ACT HWDGE ring.
  - TensorEngine reduces over H (the partition dim) with bf16 matmuls:
    stationary P_{j,k} (128 x 7) is a bf16 membership mask of the
    H-windows scaled by 1/(sh_i*sw_j), so no separate averaging pass is
    needed. Each window j owns ONE PSUM bank: all 21 matmuls (3 H-chunks
    x 7 moving-operand chunks of <=512 cols) accumulate into it, so the
    chunk dimension folds inside PSUM and windows never share banks --
    no PE<->DVE write-after-read chain between windows.
  - DVE does ONE op per window right after its stop-matmul: a strided
    X-reduce of the 512-col bank over w' straight into the output tile.
    The first six windows' results go out on sync right after window 4
    finishes; the tail window's 64 channels go out in a final small DMA.

Raw Bass blocks with explicit semaphores (TileContext's generated sync
exceeds this toolchain's per-instruction sync-wait limits).
"""

import numpy as np
import ml_dtypes

import concourse.bass as bass
import concourse.mybir as mybir
from concourse.bass_utils import run_bass_kernel_spmd

B, H, W, C = 8, 384, 384, 64
OUT = 7
N_CORES = 8
KH = H // 128  # 3 H-chunks of 128 rows
NCH = 7  # moving-operand chunks per window
# (first W column, data width, data offset inside the ring slot)
SPANS = [(0, 110, 0), (110, 110, 1), (220, 55, 1), (275, 55, 1), (329, 55, 0)]
STREAM = [4, 0, 1, 2, 3]  # span stream order (span 4 first: window 6 runs first)
NSW = 15  # slabs: 5 spans x 3 H-chunks (slab 0 goes via sync+DVE, not SWDGE)
SLOT = 111 * C  # ring slot size in elements
RING = 6  # slab ring depth
W512 = (512,) * 7
W448 = (512,) * 6 + (448,)
# (span g, view offset in slot cols, per-chunk widths)
WINDOWS = [
    (0, 0, W448),
    (0, 54, W512),
    (1, 0, W512),
    (1, 55, W512),
    (2, 0, W512),
    (3, 0, W512),
    (4, 0, W448),
]
FIRST_WIN = {4: 6, 0: 0, 1: 2, 2: 4, 3: 5}  # span -> its first window in WORDER
LAST_WIN = {4: 6, 0: 1, 1: 3, 2: 4, 3: 5}  # span -> its last window in WORDER
WORDER = [6, 0, 1, 2, 3, 4, 5]  # PE/DVE window processing order
# boundary-column patches: (src span, src slot col, dst span, WAR order idx)
PATCHES = [
    (0, 109, 1, 0),  # W109 for window 2: span0 col109 -> span1 pad, after w6
    (1, 110, 2, 1),  # W219 for window 4: span1 col110 -> span2 pad, after w0
    (2, 55, 3, 3),  # W274 for window 5: span2 col55 -> span3 pad, after w2
]
PAD_OF_WIN = {2: 0, 4: 1, 5: 2}  # window -> patch entry guarding its chunk 0

STARVE = (4, 10)  # full-starve slabs: span0 k1 (110-col) + span2 k1 (55-col)

_F32 = mybir.dt.float32
_BF16 = mybir.dt.bfloat16

SEM_TGT = [272 if _i in STARVE else 16 for _i in range(15)]
SLAB_OF = {}
for _i in range(NSW):
    SLAB_OF[(STREAM[_i // KH], _i % KH)] = _i
SPAN_OF_SLAB = [STREAM[_i // KH] for _i in range(NSW)]
SPLIT = (12, 14)  # span-3 k0/k2 slabs stream as two pieces
SPA = 4 * 512  # piece split point (view elems): chunks 0-3 | 4-6


def _windows(d, out):
    starts = np.floor(np.arange(out) * d / out).astype(np.int64)
    ends = np.ceil((np.arange(out) + 1) * d / out).astype(np.int64)
    return starts, ends - starts


def _build():
    nc = bass.Bass(num_swdge_queues=2)
    x = nc.declare_dram_parameter("x", [H, W * C], _F32, isOutput=False)
    pmat = nc.declare_dram_parameter(
        "pmat", [128, OUT * KH * OUT], _BF16, isOutput=False
    )
    out = nc.declare_dram_parameter("out", [OUT, OUT * C], _F32, isOutput=True)

    with (
        nc.sbuf_tensor([128, RING * SLOT], _BF16) as xbuf,
        nc.sbuf_tensor([128, 55 * C], _F32) as stage,
        nc.sbuf_tensor([16, 16], _F32) as shim_sb,
        nc.sbuf_tensor([128, OUT * KH * OUT], _BF16) as p_sb,
        nc.sbuf_tensor([OUT, OUT * C], _F32) as y_sb,
        nc.psum_tensor([128, OUT * 512], _F32) as psum,
        nc.semaphore("const_sem") as const_sem,
        nc.semaphore("warm_sem") as warm_sem,
        nc.semaphore("stage_sem") as stage_sem,
        nc.semaphore("pad_sem") as pad_sem,
        nc.semaphore("pe_pass_sem") as pe_pass_sem,
        nc.semaphore("win_sem") as win_sem,
        nc.semaphore("mul_sem") as mul_sem,
        nc.semaphore("out_sem") as out_sem,
    ):
        slab_sems = [nc.alloc_semaphore(f"slab{i}") for i in range(NSW)]
        pieceb_sems = {i: nc.alloc_semaphore(f"slab{i}b") for i in SPLIT}

        ORDER_OF = {j: o for o, j in enumerate(WORDER)}

        def pass_wait(eng, o, k):
            """Wait until the PE finished pass (order-index o, H-chunk k)."""
            if k == KH - 1:
                eng.wait_ge(win_sem, o + 1)
            else:
                eng.wait_ge(pe_pass_sem, o * (KH - 1) + k + 1)

        def slot_col(i, col):
            return (i % RING) * SLOT + col * C

        def slot_war(eng, i):
            """Ring-slot WAR: previous occupant's windows must be done."""
            if i >= RING:
                gp, kp = SPAN_OF_SLAB[i - RING], (i - RING) % KH
                pass_wait(eng, ORDER_OF[LAST_WIN[gp]], kp)

        with nc.Block(no_gpsimd_drain=True) as block:

            @block.gpsimd
            def _(gpsimd):
                for i in range(1, NSW):
                    g, k = SPAN_OF_SLAB[i], i % KH
                    w0, wd, doff = SPANS[g]
                    slot_war(gpsimd, i)
                    if i in STARVE:
                        # full-starve: 8x(14-runt to engines 0-13 + 2-shim
                        # to engines 14/15) + 16-row uniform; consecutive
                        # transfers alternate SWDGE queues so each engine
                        # pipelines across transfer boundaries
                        qq = [0]

                        def q_alt(dma):
                            if qq[0] % 2 == 1:
                                dma.ins.queue = "qPoolDynamic1"
                            qq[0] += 1

                        for jj in range(8):
                            r0 = 14 * jj
                            q_alt(
                                gpsimd.dma_start(
                                    out=xbuf[
                                        r0 : r0 + 14,
                                        slot_col(i, doff) : slot_col(
                                            i, doff + wd
                                        ),
                                    ],
                                    in_=x[
                                        k * 128 + r0 : k * 128 + r0 + 14,
                                        w0 * C : (w0 + wd) * C,
                                    ],
                                ).then_inc(slab_sems[i], 16)
                            )
                            q_alt(
                                gpsimd.dma_start(
                                    out=shim_sb[0:2, 0:8], in_=x[0:2, 0:8]
                                ).then_inc(slab_sems[i], 16)
                            )
                        q_alt(
                            gpsimd.dma_start(
                                out=xbuf[
                                    112:128,
                                    slot_col(i, doff) : slot_col(i, doff + wd),
                                ],
                                in_=x[
                                    k * 128 + 112 : (k + 1) * 128,
                                    w0 * C : (w0 + wd) * C,
                                ],
                            ).then_inc(slab_sems[i], 16)
                        )
                        continue
                    if i in SPLIT:
                        gpsimd.dma_start(
                            out=xbuf[
                                :, slot_col(i, doff) : slot_col(i, 0) + SPA
                            ],
                            in_=x[
                                k * 128 : (k + 1) * 128,
                                w0 * C : w0 * C + SPA - doff * C,
                            ],
                        ).then_inc(slab_sems[i], 16)
                        gpsimd.dma_start(
                            out=xbuf[
                                :,
                                slot_col(i, 0) + SPA : slot_col(i, doff)
                                + wd * C,
                            ],
                            in_=x[
                                k * 128 : (k + 1) * 128,
                                w0 * C + SPA - doff * C : (w0 + wd) * C,
                            ],
                        ).then_inc(pieceb_sems[i], 16)
                        continue
                    dma = gpsimd.dma_start(
                        out=xbuf[:, slot_col(i, doff) : slot_col(i, doff + wd)],
                        in_=x[k * 128 : (k + 1) * 128, w0 * C : (w0 + wd) * C],
                    ).then_inc(slab_sems[i], 16)
                    if i % 2 == 1:
                        dma.ins.queue = "qPoolDynamic1"

            @block.sync
            def _(sync):
                # span 4 H-chunk 0 as f32 while the SWDGE Q7 initializes
                w0, wd, _ = SPANS[4]
                sync.dma_start(
                    out=stage[:], in_=x[0:128, w0 * C : (w0 + wd) * C]
                ).then_inc(stage_sem, 16)
                # windows 0-4 and 6 are final once window 4 (order 5) scaled
                sync.wait_ge(mul_sem, 1)
                sync.dma_start(
                    out=out[:, 0 : 5 * C], in_=y_sb[:, 0 : 5 * C]
                ).then_inc(out_sem, 16)
                sync.dma_start(
                    out=out[:, 6 * C : 7 * C], in_=y_sb[:, 6 * C : 7 * C]
                ).then_inc(out_sem, 16)
                # tail window 5
                sync.wait_ge(mul_sem, 2)
                sync.dma_start(
                    out=out[:, 5 * C : 6 * C], in_=y_sb[:, 5 * C : 6 * C]
                ).then_inc(out_sem, 16)
                sync.wait_ge(out_sem, 48)

            @block.scalar
            def _(scalar):
                scalar.dma_start(out=p_sb[:], in_=pmat[:]).then_inc(const_sem, 16)
                for ent, (src_g, src_col, dst_g, war_o) in enumerate(PATCHES):
                    for k in range(KH):
                        si = SLAB_OF[(src_g, k)]
                        di = SLAB_OF[(dst_g, k)]
                        if si == 0:
                            scalar.wait_ge(warm_sem, 1)
                        else:
                            scalar.wait_ge(slab_sems[si], SEM_TGT[si])
                        # WAR: the pad column slot still holds data the
                        # previous occupant's windows read
                        pass_wait(scalar, war_o, k)
                        scalar.copy(
                            xbuf[:, slot_col(di, 0) : slot_col(di, 1)],
                            xbuf[
                                :,
                                slot_col(si, src_col) : slot_col(
                                    si, src_col + 1
                                ),
                            ],
                        ).then_inc(pad_sem, 1)

            @block.tensor
            def _(tensor):
                tensor.wait_ge(const_sem, 16)
                for o, j in enumerate(WORDER):
                    g, off, widths = WINDOWS[j]
                    for k in range(KH):
                        i = SLAB_OF[(g, k)]
                        if j == FIRST_WIN[g]:
                            if i == 0:
                                tensor.wait_ge(warm_sem, 1)
                            else:
                                tensor.wait_ge(slab_sems[i], SEM_TGT[i])
                        if j in PAD_OF_WIN:
                            tensor.wait_ge(
                                pad_sem, PAD_OF_WIN[j] * KH + k + 1
                            )
                        base = slot_col(i, off)
                        n = j * KH + k
                        lhsT = p_sb[:, n * OUT : (n + 1) * OUT]
                        for cb in range(NCH):
                            if i in SPLIT and cb == 4:
                                tensor.wait_ge(pieceb_sems[i], 16)
                            mm = tensor.matmul(
                                psum[:OUT, j * 512 : j * 512 + widths[cb]],
                                lhsT,
                                xbuf[
                                    :,
                                    base + cb * 512 : base
                                    + cb * 512
                                    + widths[cb],
                                ],
                                start=(k == 0 and cb == 0),
                                stop=(k == KH - 1 and cb == NCH - 1),
                            )
                        if k == KH - 1:
                            mm.then_inc(win_sem, 1)
                        else:
                            mm.then_inc(pe_pass_sem, 1)

            @block.vector
            def _(vector):
                # downcast the sync-streamed span4 H-chunk 0 into ring slot 0
                vector.wait_ge(stage_sem, 16)
                vector.tensor_copy(xbuf[:, 0 : 55 * C], stage[:]).then_inc(
                    warm_sem, 1
                )
                for o, j in enumerate(WORDER):
                    vector.wait_ge(win_sem, o + 1)
                    # fold the bank's 8 w' column groups into the window
                    # average (the 1/(sh*sw) scale is baked into pmat)
                    red = vector.tensor_reduce(
                        out=y_sb[:, j * C : (j + 1) * C],
                        in_=psum[:OUT, j * 512 : (j + 1) * 512].rearrange(
                            "p (w c) -> p c w", c=C
                        ),
                        axis=mybir.AxisListType.X,
                        op=mybir.AluOpType.add,
                    )
                    if o >= OUT - 2:
                        red.then_inc(mul_sem, 1)

    return nc


def _consts():
    hs, hsz = _windows(H, OUT)
    _, wsz = _windows(W, OUT)
    p = np.zeros((128, OUT * KH * OUT), np.float32)
    for j in range(OUT):
        for k in range(KH):
            n = j * KH + k
            for i in range(OUT):
                h0, h1 = int(hs[i]), int(hs[i] + hsz[i])
                for h in range(max(h0, k * 128), min(h1, (k + 1) * 128)):
                    p[h - k * 128, n * OUT + i] = 1.0 / (
                        float(hsz[i]) * float(wsz[j])
                    )
    return p.astype(ml_dtypes.bfloat16)


_NC_CACHE = None


def _run(x, **kwargs):
    global _NC_CACHE
    if _NC_CACHE is None:
        _NC_CACHE = _build()
    nc = _NC_CACHE
    p = _consts()
    x = np.ascontiguousarray(np.asarray(x, dtype=np.float32))
    in_maps = [
        {"x": x[b].reshape(H, W * C), "pmat": p}
        for b in range(N_CORES)
    ]
    res = run_bass_kernel_spmd(nc, in_maps, core_ids=list(range(N_CORES)), **kwargs)
    y = np.stack(
        [res.results[b]["out"].reshape(OUT, OUT, C) for b in range(N_CORES)]
    )
    return y, res


def kernel(x: np.ndarray) -> np.ndarray:
    y, _ = _run(x)
    return y
